# revision 43
# baseline (speedup 1.0000x reference)
"""Trainium2 Bass kernel for the DIP module (tone curve + white balance +
contrast-about-mean + 3x3 sharpen blend), data-parallel over batch on 8 cores.

v3 design (from trace analysis of the v2 kernel at 152us):
  v2's trace showed the PE spending 80% of the kernel at HAM K=4/8 (half
  clock): recurring 1.4-1.7us PE stalls once per channel-PAIR (the batched
  two-channel Ln op created a 3.7us serial ACT bubble) tripped the HAM idle
  window, and each trip costs ~16us of half-rate matmuls. The warm-up
  matmul block also wasted ~10k PE columns and was followed by a 14us
  pipeline-fill idle that re-throttled the PE anyway.
  - per-channel Ln (no pairing): evens out the ACT pipeline so the PE's
    uc feed never bubbles at pair boundaries.
  - no warm-up block; instead tiny N=64 keep-alive matmuls are issued at
    the known PE stall points (before each mean-matmul) so the HAM
    activity window never sees the PE idle.
  - fp16 staging of x and y in DRAM (host casts): halves HBM traffic.
    Host pre-tiles x to [IPC, C, 128, NT*512] so every DMA is a plain
    per-partition linear copy with 2-4KB contiguous runs.
  - ACT: Ln then Exp(scale=g, bias=ln(a*wb), accum_out=row sums), fp16 out.
  - DVE: u = clip01(t' + cb) in ONE fp16 tensor_scalar (a<=1 slots; two
    ops for a>1 slots), halo 3-sums on zero-padded tiles (2 TT ops, no
    edge copy), final clip01 PSUM->fp16 in two [128,1024] ops per channel.
  - conv folds the (1+8s)*u center term into the mid matmul (no DVE add):
    out = (1+8s)u - s*(8 neighbors) accumulated in PSUM by 4 fp16 matmuls
    per 128-row tile, issued grouped BY WEIGHT MATRIX (mid x2, side x4,
    halo x2 per 2-tile group) so LDWEIGHTS is reusable.
  - DMA dispatch split: x loads + halo + stores round-robin on sync/gpsimd
    HWDGE queues; the scalar engine only runs ACT ops; const loads go on
    scalar before Ln0.
"""

import numpy as np

try:
    import concourse.bass as bass
except ImportError:  # pragma: no cover
    import sys

    sys.path.insert(0, "/opt/trn_rl_repo")
    import concourse.bass as bass

from contextlib import ExitStack

import concourse.bacc as bacc
import concourse.tile as tile
from concourse import mybir
from concourse.bass_utils import run_bass_kernel_spmd

F32 = mybir.dt.float32
F16 = mybir.dt.float16
U8 = mybir.dt.uint8


B, C, H, W = 32, 3, 512, 512
NCORES = 8
IPC = B // NCORES  # images per core
NT = H // 128  # row tiles per channel
NPIX = H * W
FD = NT * W  # free-dim elements per partition per channel (2048)


class _Bacc(bacc.Bacc):
    """Bacc that (a) pins Exp/Ln to the combined table set so the kernel does
    a single ACT_TABLE_LOAD instead of thrashing between exp/ln sets, and
    (b) elides redundant InstLdweights: conv matmuls are issued grouped by
    weight matrix, so back-to-back matmuls can reuse the PE's stationary
    weights instead of paying a ~340ns reload gap each (which also keeps the
    PE's HAM activity window saturated -> 2.4 GHz clock)."""

    def _dedup_ldweights(self):
        def sig_of(inst):
            a = inst.ins[0]
            return (a.memref, a.offset, str(a.dtype), str(a.ap))

        def sync_empty(inst):
            si = inst.sync_info
            return si is None or (len(si.on_wait) == 0 and len(si.on_update) == 0)

        for blk in self.main_func.blocks:
            last_sig = None
            kept = []
            for inst in blk.instructions:
                if isinstance(inst, mybir.InstLdweights):
                    sig = sig_of(inst)
                    if sig == last_sig and sync_empty(inst):
                        continue
                    last_sig = sig
                elif isinstance(inst, (mybir.InstMatmult, mybir.InstMatmultMx)):
                    w = inst.ins[1] if len(inst.ins) > 1 else None
                    if w is None or str(getattr(w, "dtype", "")) in (
                        "dt.float32",
                        "dt.float32r",
                    ):
                        # fp32 matmuls self-load and clobber the PE weights
                        last_sig = None
                elif not isinstance(
                    inst,
                    (
                        mybir.InstEventSemaphore,
                        mybir.InstDMACopy,
                        mybir.InstTensorScalarPtr,
                        mybir.InstTensorTensor,
                        mybir.InstTensorCopy,
                        mybir.InstActivation,
                        mybir.InstMemset,
                        mybir.InstDrain,
                    ),
                ):
                    # unknown instruction (branch, call, ...): be conservative
                    last_sig = None
                kept.append(inst)
            blk.instructions[:] = kept

    def compile(self):
        self._dedup_ldweights()
        super().compile()

    def insert_act_table_loads(self):
        import bass_rust as _bass_rust

        from concourse.hw_specs import get_activation_tables

        has_activation = any(
            isinstance(i, mybir.InstActivation)
            for b in self.main_func.blocks
            for i in b.instructions
        )
        if not has_activation:
            return
        AF = mybir.ActivationFunctionType
        tables = []
        for name, funcs in get_activation_tables(self.m.arch).items():
            if name != "natural_log_exp_and_others":
                funcs = funcs - {AF.Exp, AF.Ln}
            tables.append((name, funcs))
        _bass_rust.insert_act_table_loads(self, tables)


def _build_program(slotmask):
    nc = _Bacc("TRN2", target_bir_lowering=False)

    # x pre-tiled on host: x_in[i, c, p, k*512 + j] = x[img, c, 128*k + p, j],
    # staged as uint8 (Ln's free affine rescales by 1/255): halves load HBM
    x_in = nc.declare_dram_parameter("x_in", [IPC, C, 128, FD], U8, isOutput=False)
    # [K row, image, {side, mid}, M row]
    mats = nc.declare_dram_parameter("mats", [128, IPC, 2, 128], F16, isOutput=False)
    # [halo idx, image, tile, M row]
    hmats = nc.declare_dram_parameter("hmats", [8, IPC, NT, 128], F16, isOutput=False)
    emat = nc.declare_dram_parameter("emat", [128, IPC, 128], F32, isOutput=False)
    gcol = nc.declare_dram_parameter("gcol", [128, IPC], F32, isOutput=False)
    scal = nc.declare_dram_parameter("scal", [128, IPC * C], F32, isOutput=False)
    # y staged as uint8 (saturating cast implements the upper clip; host
    # divides by 255): halves the store-side HBM traffic
    y_out = nc.declare_dram_parameter("y_out", [IPC, C, 128, FD], U8, isOutput=True)

    AF = mybir.ActivationFunctionType
    ALU = mybir.AluOpType

    with ExitStack() as ctx:
        tc = ctx.enter_context(tile.TileContext(nc))
        singles = ctx.enter_context(tc.tile_pool(name="singles", bufs=1))
        xp = ctx.enter_context(tc.tile_pool(name="xp", bufs=2))
        lnp = ctx.enter_context(tc.tile_pool(name="lnp", bufs=2))
        tpp = ctx.enter_context(tc.tile_pool(name="tpp", bufs=4))
        w1p = ctx.enter_context(tc.tile_pool(name="w1p", bufs=2))
        up = ctx.enter_context(tc.tile_pool(name="up", bufs=3))
        accp = ctx.enter_context(tc.tile_pool(name="accp", bufs=3))
        cbpp = ctx.enter_context(tc.tile_pool(name="cbpp", bufs=1, space="PSUM"))
        cbp = ctx.enter_context(tc.tile_pool(name="cbp", bufs=3))
        halop = ctx.enter_context(tc.tile_pool(name="halop", bufs=3))
        hsump = ctx.enter_context(tc.tile_pool(name="hsump", bufs=3))
        # one conv PSUM pool, triple-buffered (6 banks): each [128,2,512]
        # fp32 tile has ~1.5 channels of slack before its bank is needed
        # again, so the clip01 drains never gate the conv matmuls
        outpp = ctx.enter_context(tc.tile_pool(name="outpp", bufs=3, space="PSUM"))
        outsp = ctx.enter_context(tc.tile_pool(name="outsp", bufs=3))
        # dedicated keep-alive PSUM bank: no readers and no cross-engine
        # deps, so fillers run exactly in the PE's dependency-wait windows
        kap = ctx.enter_context(tc.tile_pool(name="kap", bufs=1, space="PSUM"))

        # ---- constants into SBUF (scalar queue: done before Ln0 needs them) ----
        mats_sb = singles.tile([128, IPC, 2, 128], F16)
        nc.scalar.dma_start(out=mats_sb[:, :, :, :], in_=mats[:, :, :, :])
        hmats_sb = singles.tile([8, IPC, NT, 128], F16)
        nc.scalar.dma_start(out=hmats_sb[:, :, :, :], in_=hmats[:, :, :, :])
        emat_sb = singles.tile([128, IPC, 128], F32)
        nc.scalar.dma_start(out=emat_sb[:, :, :], in_=emat[:, :, :])
        gcol_sb = singles.tile([128, IPC], F32)
        nc.scalar.dma_start(out=gcol_sb[:, :], in_=gcol[:, :])
        scal_sb = singles.tile([128, IPC * C], F32)
        nc.scalar.dma_start(out=scal_sb[:, :], in_=scal[:, :])

        # ---- memset-backed tiles: keep-alive matmul operand (no DMA dep) and
        # zero-padded clipped-halo tiles (cols 0 and 513 stay 0 forever) ----
        wmm_in = singles.tile([128, 512], F16)
        nc.vector.memset(wmm_in[:, :], 0.5)
        for _ in range(3):
            hz = halop.tile([8, 514], F16, tag="hu")
            nc.vector.memset(hz[:, :], 0.0)

        # ---- HAM keep-alive: tiny matmuls issued right before known PE
        # stall points so the HAM activity window never sees the PE idle
        # (an idle window drops the PE to K=4/8 for ~16us) ----
        def keepalive(n, cols=512, rhs=None):
            wps = kap.tile([128, 512], F32, tag="ka")
            if rhs is None:
                rhs = wmm_in
            for _ in range(n):
                nc.tensor.matmul(
                    out=wps[:, 0:cols],
                    lhsT=wmm_in[:, 0:128],
                    rhs=rhs[:, 0:cols],
                    start=True,
                    stop=True,
                )

        def phase_a1(i, c, halves=1):
            # one load dispatch per channel (sync queue is loads-only so the
            # prefetch stream is never blocked behind compute-dependent DMAs);
            # per-channel Ln keeps the ACT pipeline fine-grained so the PE's
            # uc feed never bubbles. halves=2 chunks the load/Ln/Exp so the
            # first channel's pipeline starts on the first half-load.
            ch = i * C + c
            st = {"i": i, "c": c}
            xb = xp.tile([128, NT, 512], U8)
            lt = lnp.tile([128, NT, 512], F16)
            acc = accp.tile([128, halves], F32)
            tpc = tpp.tile([128, NT, 512], F16)
            xsrc = x_in[i, c, :, :].rearrange("p (k j) -> p k j", k=NT)
            hk = NT // halves
            for h in range(halves):
                sl = slice(h * hk, (h + 1) * hk)
                nc.sync.dma_start(out=xb[:, sl, :], in_=xsrc[:, sl, :])
                nc.scalar.activation(
                    out=lt[:, sl, :], in_=xb[:, sl, :], func=AF.Ln, scale=1.0 / 255.0
                )
                nc.scalar.activation(
                    out=tpc[:, sl, :],
                    in_=lt[:, sl, :],
                    func=AF.Exp,
                    scale=gcol_sb[:, i : i + 1],
                    bias=scal_sb[:, ch : ch + 1],
                    accum_out=acc[:, h : h + 1],
                )
            st["halves"] = halves
            st["tpc"], st["acc"] = tpc, acc
            st["xb"], st["lt"] = xb, lt
            return st

        def phase_a2(st):
            # ---- mean -> cb column: cb[m] = const * sum_p,h acc[p,h] [PE] ----
            i, halves = st["i"], st["halves"]
            cbps = cbpp.tile([128, 1], F32, tag="cbps")
            for h in range(halves):
                nc.tensor.matmul(
                    out=cbps[:, :],
                    lhsT=emat_sb[:, i, :],
                    rhs=st["acc"][:, h : h + 1],
                    start=(h == 0),
                    stop=(h == halves - 1),
                )
            st["cbps"] = cbps

        def uclip_half(st, sl):
            # u = clip01(t' + cb) for tiles in slice sl
            i, tpc, cb, uc = st["i"], st["tpc"], st["cb"], st["uc"]
            if slotmask[i]:
                # a > 1 -> cb < 0: need the max(.,0)
                w1c = w1p.tile([128, 2, 512], F16)
                nc.vector.tensor_scalar(
                    w1c[:, :, :], tpc[:, sl, :], cb[:, 0:1], 0.0, ALU.add, ALU.max
                )
                nc.vector.tensor_scalar(uc[:, sl, :], w1c[:, :, :], 1.0, None, ALU.min)
            else:
                # a <= 1 -> cb >= 0 and t' >= 0: max(.,0) is a no-op
                nc.vector.tensor_scalar(
                    uc[:, sl, :], tpc[:, sl, :], cb[:, 0:1], 1.0, ALU.add, ALU.min
                )

        def phase_a3(st):
            # ---- DVE/sync work for this channel, in the order the PE needs
            # it: cb copy, u-clip of group B (tiles 2,3), halo clip + 3-sum
            # (needed by B's trailing halo matmuls), u-clip of group A ----
            i, tpc = st["i"], st["tpc"]
            # halo rows: one DMA, rows 0..3 = row 0 of tiles 0..3, rows
            # 4..7 = row 127 of tiles 0..3 (sync queue: the gpsimd queue's
            # store dispatches would delay hs past the halo matmuls)
            th = hsump.tile([8, 512], F16, tag="th")
            nc.sync.dma_start(out=th[0:8, :], in_=tpc[0:128:127, :, :])
            cb = cbp.tile([128, 1], F32)
            nc.vector.tensor_copy(out=cb[:, :], in_=st["cbps"][:, :])
            st["cb"] = cb
            uc = up.tile([128, NT, 512], F16)
            st["uc"] = uc
            uclip_half(st, slice(2, 4))
            hu = halop.tile([8, 514], F16, tag="hu")
            if slotmask[i]:
                w1h = hsump.tile([8, 512], F16, tag="w1h")
                nc.vector.tensor_scalar(
                    w1h[:, :], th[:, :], cb[0:8, 0:1], 0.0, ALU.add, ALU.max
                )
                nc.vector.tensor_scalar(hu[:, 1:513], w1h[:, :], 1.0, None, ALU.min)
            else:
                nc.vector.tensor_scalar(
                    hu[:, 1:513], th[:, :], cb[0:8, 0:1], 1.0, ALU.add, ALU.min
                )
            hpair = hsump.tile([8, 512], F16, tag="hpair")
            hs = hsump.tile([8, 512], F16, tag="hs")
            nc.vector.tensor_add(hpair[:, :], hu[:, 0:512], hu[:, 1:513])
            nc.vector.tensor_add(hs[:, :], hpair[:, :], hu[:, 2:514])
            st["hs"] = hs
            uclip_half(st, slice(0, 2))

        def conv_group(st, ob, ks, ka_pre_halo=0):
            i = st["i"]
            uc, hs = st["uc"], st["hs"]
            mmid = mats_sb[:, i, 1, :]
            mside = mats_sb[:, i, 0, :]
            for kk, k in enumerate(ks):
                nc.tensor.matmul(
                    out=ob[:, kk, 0:512],
                    lhsT=mmid,
                    rhs=uc[:, k, 0:512],
                    start=True,
                    stop=False,
                )
            for kk, k in enumerate(ks):
                nc.tensor.matmul(
                    out=ob[:, kk, 1:512],
                    lhsT=mside,
                    rhs=uc[:, k, 0:511],
                    start=False,
                    stop=False,
                )
                nc.tensor.matmul(
                    out=ob[:, kk, 0:511],
                    lhsT=mside,
                    rhs=uc[:, k, 1:512],
                    start=False,
                    stop=False,
                )
            for kk, k in enumerate(ks):
                nc.tensor.matmul(
                    out=ob[:, kk, 0:512],
                    lhsT=hmats_sb[0:8, i, k, :],
                    rhs=hs[:, 0:512],
                    start=False,
                    stop=True,
                )

        def phase_b_mms(st):
            # ---- conv matmuls grouped by weight matrix, halos last. The
            # clip01 of group B is issued between the groups (it drains
            # while group A runs); group A's clip is deferred past the next
            # channel's u-clips (phase_b_finA). ----
            obB = outpp.tile([128, 2, 512], F32, tag="ob")
            obA = outpp.tile([128, 2, 512], F32, tag="ob")
            oc = outsp.tile([128, NT, 512], U8)
            st["obA"], st["oc"] = obA, oc
            conv_group(st, obB, (2, 3))
            # clip01 * 255 -> uint8: the saturating cast supplies the min(.,1)
            nc.vector.tensor_scalar(
                oc[:, 2:4, :], obB[:, :, :], 0.0, 255.0, ALU.max, ALU.mult
            )
            conv_group(st, obA, (0, 1))

        def phase_b_finA(st):
            # ---- clip01 of group A (deferred past the next channel's
            # u-clips on the DVE queue; obA is double-buffered), store ----
            i, c = st["i"], st["c"]
            oc = st["oc"]
            nc.vector.tensor_scalar(
                oc[:, 0:2, :], st["obA"][:, :, :], 0.0, 255.0, ALU.max, ALU.mult
            )
            ydst = y_out[i, c, :, :].rearrange("p (k j) -> p k j", k=NT)
            nc.gpsimd.dma_start(out=ydst[:, :, :], in_=oc[:, :, :])

        def process(st, prev):
            # keep-alives bridge the PE's two dependency waits: the Exp
            # accumulator (mean matmul input) and the cb/u-clip DVE chain
            keepalive(1)
            phase_a2(st)
            keepalive(3 if slotmask[st["i"]] else 2)
            phase_a3(st)
            if prev is not None:
                phase_b_finA(prev)
            phase_b_mms(st)

        chans = [(i, c) for i in range(IPC) for c in range(C)]
        prev = None
        cur = None
        for idx, (i, c) in enumerate(chans):
            nxt = phase_a1(i, c, halves=2 if idx == 0 else 1)
            if idx == 1:
                # bridge the startup pipeline-fill window: keep-alives whose
                # rhs depends on progressively later loads/ACT outputs, so
                # the PE shows activity every <2us until the first conv.
                # (uint8 xb can't feed an fp16 matmul directly; bounce each
                # load-completion anchor through a tiny DVE copy)
                keepalive(2, rhs=mats_sb[:, 0, 0, :], cols=128)
                for src in (cur["xb"][:, 0, 0:128], cur["xb"][:, 2, 0:128],
                            nxt["xb"][:, 0, 0:128]):
                    anc = w1p.tile([128, 128], F16, tag="anc")
                    nc.vector.tensor_copy(out=anc[:, :], in_=src)
                    keepalive(2, rhs=anc, cols=128)
                keepalive(2, rhs=cur["lt"][:, 0, :])
                keepalive(2, rhs=cur["lt"][:, 2, :])
            if cur is not None:
                process(cur, prev)
                prev = cur
            cur = nxt
        process(cur, prev)
        phase_b_finA(cur)
    nc.compile()
    return nc


def _host_inputs(x, gamma, wb, contrast, sharpen_strength, idx):
    """Build per-core input maps (numpy only). idx[cid][i] = global image."""
    eye0 = np.eye(128, dtype=np.float32)
    tri = eye0 + np.eye(128, k=1, dtype=np.float32) + np.eye(128, k=-1, dtype=np.float32)
    in_maps = []
    for cid in range(NCORES):
        imgs = idx[cid]
        mats = np.zeros((128, IPC, 2, 128), np.float16)
        hmats = np.zeros((8, IPC, NT, 128), np.float16)
        emat = np.zeros((128, IPC, 128), np.float32)
        gcol = np.zeros((128, IPC), np.float32)
        scal = np.zeros((128, IPC * C), np.float32)
        for i in range(IPC):
            b = imgs[i]
            a = float(contrast[b])
            s = float(sharpen_strength[b])
            g = float(gamma[b])
            # side = tridiag(-s); mid = side + (1+9s) on the diagonal
            # (center tap (1+8s) = -s + (1+9s))
            mats[:, i, 0, :] = (-s * tri).astype(np.float16)
            mats[:, i, 1, :] = (-s * tri + (1.0 + 9.0 * s) * eye0).astype(np.float16)
            # halo: tile k's top neighbor (row 127 of tile k-1) is halo row
            # 3+k -> out row 0; bottom neighbor (row 0 of tile k+1) is halo
            # row k+1 -> out row 127
            for k in range(NT):
                if k >= 1:
                    hmats[3 + k, i, k, 0] = -s
                if k <= 2:
                    hmats[k + 1, i, k, 127] = -s
            emat[:, i, :] = (1.0 - a) / (a * NPIX)
            gcol[:, i] = g
            for c in range(C):
                scal[:, i * C + c] = np.log(a * float(wb[b, c]))
        # pre-tile x: [i, c, 128k+p, j] -> [i, c, p, k*512+j], quantize u8
        xs = x[imgs].reshape(IPC, C, NT, 128, W).transpose(0, 1, 3, 2, 4)
        xs = np.ascontiguousarray(xs).reshape(IPC, C, 128, FD)
        xs = np.rint(xs * 255.0).astype(np.uint8)
        in_maps.append(
            {
                "x_in": xs,
                "mats": mats,
                "hmats": hmats,
                "emat": emat,
                "gcol": gcol,
                "scal": scal,
            }
        )
    return in_maps


_PROGRAM_CACHE = {}


def kernel(x, gamma, wb, contrast, sharpen_strength):
    x = np.asarray(x, dtype=np.float32)
    gamma = np.asarray(gamma, dtype=np.float32)
    wb = np.asarray(wb, dtype=np.float32)
    contrast = np.asarray(contrast, dtype=np.float32)
    sharpen_strength = np.asarray(sharpen_strength, dtype=np.float32)

    # Sort images by contrast and stripe across cores so slot i is
    # homogeneous in sign(1-a); the single-op clip path is only legal
    # when every image in the slot has a <= 1 (SPMD: shared program).
    order = np.argsort(contrast, kind="stable")
    idx = [[int(order[i * NCORES + cid]) for i in range(IPC)] for cid in range(NCORES)]
    slotmask = tuple(
        bool(any(contrast[order[i * NCORES + cid]] > 1.0 for cid in range(NCORES)))
        for i in range(IPC)
    )
    if slotmask not in _PROGRAM_CACHE:
        _PROGRAM_CACHE.clear()
        _PROGRAM_CACHE[slotmask] = _build_program(slotmask)
    nc = _PROGRAM_CACHE[slotmask]

    in_maps = _host_inputs(x, gamma, wb, contrast, sharpen_strength, idx)
    res = run_bass_kernel_spmd(nc, in_maps, list(range(NCORES)))
    out = np.empty((B, C, H, W), np.float32)
    for cid in range(NCORES):
        for i in range(IPC):
            yt = res.results[cid]["y_out"][i].astype(np.float32) * (1.0 / 255.0)
            # un-tile: [c, p, k*512+j] -> [c, 128k+p, j]
            yt = yt.reshape(C, 128, NT, W).transpose(0, 2, 1, 3).reshape(C, H, W)
            out[idx[cid][i]] = yt
    return out

